# revision 8
# baseline (speedup 1.0000x reference)
"""Multi-head self-attention (B=2, N=4096, C=384, H=6) on 8 Trainium2 NeuronCores.

Sharding: core = (batch, query-quarter). Each core recomputes K/V for its batch
from x (no cross-core communication), computes Q for its 1024 query rows, runs
softmax(Q K^T / sqrt(D)) V for all 6 heads, and applies the output projection
for its rows. Host concatenates the 8 row-slices.

Key device-side choices:
  - Everything is kept "transposed" (channel on partitions) so the PE never
    needs an on-chip transpose.
  - Heads are processed in pairs; the two K=64 score matmuls of a pair are
    row-tiled (partitions 0:64 / 64:128) so they can run concurrently in
    different PE row-groups.
  - softmax exp is computed as a Schraudolph bit-trick on THREE engines
    (ACT / DVE / Pool round-robin over score groups):
        i16 = round_i16(s * (log2e*128) + B16);  bitcast(i16) == bf16(~e^s)
    Max per-element error ~3.3%, but the error is a deterministic function of
    frac(s*log2e) shared by numerator and denominator of the softmax, so the
    end-to-end output error stays ~8e-3 (measured vs fp64 reference).
    This frees the ACT engine (the old exp bottleneck) to 1/3 of the work.
  - V carries an appended ones-column per head, so the P@V matmul also
    produces the softmax denominator for free (row 64 of O^T).
  - V and exp(S) are bf16 operands; scores stay f32r (Q/K^T f32r).
  - O accumulators are copied PSUM->SBUF (Pool) immediately after the last
    accumulation so the PSUM banks recycle fast; the normalization chain
    (DVE reciprocal + DRAM-bounce partition-broadcast + DVE multiply) runs
    entirely off the PE queue.
  - The normalized per-head outputs are written into per-PAIR [128, QPC]
    tiles so the output projection contracts K=128 (3 accumulation steps
    instead of 6).
"""

import numpy as np
from contextlib import ExitStack

import concourse.bass as bass
import concourse.bacc as bacc
import concourse.tile as tile
from concourse import mybir
from concourse.bass_utils import run_bass_kernel_spmd

B, N, C = 2, 4096, 384
H, D = 6, 64
SCALE = D ** -0.5
P = 128
QPC = 1024          # query rows per core
NCORES = 8
PAIRS = H // 2      # 3 head pairs
NT = N // P         # 32 k-tiles
MDT = mybir.dt.float32r
F32 = mybir.dt.float32
BF16 = mybir.dt.bfloat16
I16 = mybir.dt.int16
EXPC = mybir.ActivationFunctionType.Copy

LOG2E = float(np.log2(np.e))
A16 = LOG2E * 128.0
B16 = 16250.375
# exp-engine round-robin over score groups: A=ACT, D=DVE (Pool cannot read
# PSUM, so only these two engines can consume score tiles). ~4:3 matches the
# engines' effective elem rates (ACT ~126 G/s vs DVE ~110 G/s incl. its
# normalization work).
EXP_PATTERN = "ADAADAD"


def _emit(ctx: ExitStack, tc, nc, xT, xqT, wq, wk, wv, wp2, bias, out):
    sing = ctx.enter_context(tc.tile_pool(name="sing", bufs=1))
    ktp = ctx.enter_context(tc.tile_pool(name="ktp", bufs=1))
    vp = ctx.enter_context(tc.tile_pool(name="vp", bufs=1))
    prep = ctx.enter_context(tc.tile_pool(name="prep", bufs=1))
    outp = ctx.enter_context(tc.tile_pool(name="outp", bufs=2))
    drp = ctx.enter_context(tc.tile_pool(name="drp", bufs=4, space="DRAM"))
    # PSUM pools are phase-scoped (stack discipline): kvgen/Q-gen use psp,
    # attention uses ssp (3 score bufs) + osp, proj re-creates a small pool.
    pctx = ExitStack()
    psp = pctx.enter_context(tc.tile_pool(name="psp", bufs=2, space="PSUM"))
    # created last / released right after Q^T generation (stack discipline)
    qctx = ExitStack()
    qtmp = qctx.enter_context(tc.tile_pool(name="qtmp", bufs=1))

    # ---- load weights / per-core query slice ----
    wq_sb = qtmp.tile([P, 3, C], MDT, name="wq_sb")
    wk_sb = sing.tile([P, 3, C], MDT, name="wk_sb")
    wv_sb = sing.tile([P, 3, C], MDT, name="wv_sb")
    wp_sb = sing.tile([P, PAIRS, C], MDT, name="wp_sb")
    xq_sb = qtmp.tile([P, 3, QPC], MDT, name="xq_sb")
    nc.sync.dma_start(out=wq_sb, in_=wq[:, :, :])
    nc.sync.dma_start(out=wk_sb, in_=wk[:, :, :])
    nc.sync.dma_start(out=wv_sb, in_=wv[:, :, :])
    nc.sync.dma_start(out=wp_sb, in_=wp2[:, :, :])
    nc.sync.dma_start(
        out=xq_sb, in_=xqT[:, :].rearrange("(ck p) q -> p ck q", p=P)
    )

    bias_bc = sing.tile([P, C], F32, name="bias_bc")
    b_ap = bias[:, :]
    nc.sync.dma_start(
        out=bias_bc, in_=bass.AP(b_ap.tensor, b_ap.offset, [[0, P], [1, C]])
    )

    # ---- Q^T for all pairs: QT[:, pair, q] = (wq_pair)^T @ xq ----
    qt_sb = sing.tile([P, PAIRS, QPC], MDT, name="qt_sb")
    for pair in range(PAIRS):
        for qt in range(QPC // 512):
            ps = psp.tile([P, 512], F32, name="ps")
            for ck in range(3):
                nc.tensor.matmul(
                    ps,
                    lhsT=wq_sb[:, ck, pair * 128:(pair + 1) * 128],
                    rhs=xq_sb[:, ck, qt * 512:(qt + 1) * 512],
                    start=(ck == 0),
                    stop=(ck == 2),
                )
            nc.vector.tensor_copy(qt_sb[:, pair, qt * 512:(qt + 1) * 512], ps)
    qctx.close()  # wq/xq SBUF space is no longer needed
    # these pools reuse the released qtmp space (created after the pop so the
    # stack allocator can place them there)
    xchp = ctx.enter_context(tc.tile_pool(name="xchp", bufs=3))
    expp = ctx.enter_context(tc.tile_pool(name="expp", bufs=4))
    rbp = ctx.enter_context(tc.tile_pool(name="rbp", bufs=3))

    # per-PAIR normalized outputs, heads stacked on partitions (proj K=128)
    pre2 = [prep.tile([P, QPC], MDT, name=f"pre{p}") for p in range(PAIRS)]

    xT_r = xT[:, :].rearrange("(ck p) n -> p ck n", p=P)

    # ---- V for ALL pairs (with ones columns), one xT streaming pass ----
    v_all = vp.tile([P, PAIRS, NT, 130], BF16, name="v_all")
    nc.vector.memset(v_all[:, :, :, 64:65], 1.0)
    nc.vector.memset(v_all[:, :, :, 129:130], 1.0)

    def kvgen(kt_tiles):
        """One xT streaming pass computing V (all pairs) and K^T (all pairs).
        PE-bound (~3.8us/chunk vs ~2.4us DMA), so prefetch hides the DMA.
        V copies ride the Pool engine, K^T copies the DVE."""
        for nt8 in range(N // 512):
            xch = xchp.tile([P, 3, 512], MDT, name="xch")
            nc.sync.dma_start(out=xch, in_=xT_r[:, :, nt8 * 512:(nt8 + 1) * 512])
            for sub in range(4):
                nt = nt8 * 4 + sub
                psv = psp.tile([P, 512], F32, name="ps")[:, 0:C]
                for ck in range(3):
                    nc.tensor.matmul(
                        psv,
                        lhsT=xch[:, ck, sub * 128:(sub + 1) * 128],
                        rhs=wv_sb[:, ck, :],
                        start=(ck == 0),
                        stop=(ck == 2),
                    )
                # one copy for all pairs: psv [3 pairs x 2 heads x 64] ->
                # v_all column blocks (0:64 / 65:129 per pair). On ACT: it is
                # idle during kvgen and Pool cannot read PSUM.
                nc.scalar.copy(
                    v_all[:, :, nt, 0:130]
                    .rearrange("p r (two x) -> p r two x", two=2)[:, :, :, 0:64],
                    psv.rearrange("p (r two x) -> p r two x", r=PAIRS, two=2),
                )
            for p in range(PAIRS):
                ps = psp.tile([P, 512], F32, name="ps")
                for ck in range(3):
                    nc.tensor.matmul(
                        ps,
                        lhsT=wk_sb[:, ck, p * 128:(p + 1) * 128],
                        rhs=xch[:, ck, :],
                        start=(ck == 0),
                        stop=(ck == 2),
                    )
                nc.vector.tensor_copy(
                    kt_tiles[p][:, nt8 * 512:(nt8 + 1) * 512], ps
                )

    GRP = 2  # score chunks (512 q-cols each) per PSUM score tile / exp op

    def exp_op(idx, e_t, s_t, nelem):
        eng = EXP_PATTERN[idx % len(EXP_PATTERN)]
        if eng == "A":
            nc.scalar.activation(
                e_t[:, 0:nelem], s_t[:, 0:nelem], EXPC, bias=B16, scale=A16
            )
        else:
            nc.vector.tensor_scalar(
                e_t[:, 0:nelem], s_t[:, 0:nelem], A16, B16,
                mybir.AluOpType.mult, mybir.AluOpType.add,
            )

    def attention(pair, kt_t):
        NCH = 2 * NT  # chunk c = (kt = c//2, half = c%2)

        def s_chunk(s_t, j, c, qt):
            kt, half = c // 2, c % 2
            lo = 64 * half
            nc.tensor.matmul(
                s_t[:, j * 512:(j + 1) * 512],
                lhsT=kt_t[lo:lo + 64, kt * 128:(kt + 1) * 128],
                rhs=qt_sb[lo:lo + 64, pair, qt * 512:(qt + 1) * 512],
                start=True,
                stop=True,
            )

        def s_group(g, qt):
            nch = min(GRP, NCH - g * GRP)
            s_t = ssp.tile([P, GRP * 512], F32, name="s")
            for j in range(nch):
                s_chunk(s_t, j, g * GRP + j, qt)
            return s_t, nch

        for qt in range(QPC // 512):
            # alternate accumulator banks by qt parity so the next q-tile's
            # P@V can start while this one's normalization chain still reads
            o_e = osp.tile([65, 512], F32, name=f"oe{qt % 2}")
            o_o = osp.tile([65, 512], F32, name=f"oo{qt % 2}")
            ngrp = (NCH + GRP - 1) // GRP
            # score matmuls run two groups ahead of exp (3 PSUM score bufs)
            s_ts = [s_group(0, qt), s_group(1, qt)]
            for g in range(ngrp):
                s_t, nch = s_ts.pop(0)
                e_t = expp.tile([P, GRP * 512], I16, name="etile")
                exp_op(pair * 64 + qt * 32 + g, e_t, s_t, nch * 512)
                if g + 2 < ngrp:
                    s_ts.append(s_group(g + 2, qt))
                for j in range(nch):
                    c = g * GRP + j
                    kt, half = c // 2, c % 2
                    nc.tensor.matmul(
                        o_o if half else o_e,
                        lhsT=v_all[:, pair, kt, 65:130] if half
                        else v_all[:, pair, kt, 0:65],
                        rhs=e_t[:, j * 512:(j + 1) * 512].bitcast(BF16),
                        start=(kt == 0),
                        stop=(kt == NT - 1),
                    )
            # normalize: pre2[pair][h*64:(h+1)*64, q] = O^T[0:64, q] / O^T[64, q]
            for hh, o_t in ((0, o_e), (1, o_o)):
                recip = rbp.tile([1, 512], F32, name="recip")
                nc.vector.reciprocal(recip, o_t[64:65, :])
                # partition-broadcast via a DRAM bounce (step-0 partition APs
                # are only legal on DRAM); keeps the normalization chain off
                # the PE queue and PSUM, overlapped thanks to the o buffers
                rdr = drp.tile([1, 512], F32, name="rdr")
                nc.sync.dma_start(out=rdr, in_=recip)
                rb_sb = rbp.tile([64, 512], F32, name="rb")
                rap = rdr[:, :]
                nc.sync.dma_start(
                    out=rb_sb,
                    in_=bass.AP(rap.tensor, rap.offset, [[0, 64], [1, 512]]),
                )
                nc.vector.tensor_mul(
                    pre2[pair][hh * 64:(hh + 1) * 64, qt * 512:(qt + 1) * 512],
                    o_t[0:64, :], rb_sb
                )

    kt_tiles = [ktp.tile([P, N], MDT, name=f"kt{pair}") for pair in range(PAIRS)]
    kvgen(kt_tiles)
    pctx.close()  # free kvgen PSUM banks for the attention pools

    actx = ExitStack()
    ssp = actx.enter_context(tc.tile_pool(name="ssp", bufs=2, space="PSUM"))
    osp = actx.enter_context(tc.tile_pool(name="osp", bufs=1, space="PSUM"))
    for pair in range(PAIRS):
        attention(pair, kt_tiles[pair])
    actx.close()

    psp = ctx.enter_context(tc.tile_pool(name="psp2", bufs=2, space="PSUM"))

    # ---- output projection: contract K=128 over head pairs ----
    for qc in range(QPC // P):
        ps = psp.tile([P, 512], F32, name="ps")[:, 0:C]
        for pair in range(PAIRS):
            nc.tensor.matmul(
                ps,
                lhsT=pre2[pair][:, qc * P:(qc + 1) * P],
                rhs=wp_sb[:, pair, :],
                start=(pair == 0),
                stop=(pair == PAIRS - 1),
            )
        o_sb = outp.tile([P, C], F32, name="osb")
        nc.vector.tensor_add(o_sb, ps, bias_bc)
        nc.sync.dma_start(out=out[qc * P:(qc + 1) * P, :], in_=o_sb)


def build_nc(reps=1):
    nc = bacc.Bacc()
    xT = nc.dram_tensor("xT", [C, N], MDT, kind="ExternalInput")
    xqT = nc.dram_tensor("xqT", [C, QPC], MDT, kind="ExternalInput")
    wq = nc.dram_tensor("wq", [P, 3, C], MDT, kind="ExternalInput")
    wk = nc.dram_tensor("wk", [P, 3, C], MDT, kind="ExternalInput")
    wv = nc.dram_tensor("wv", [P, 3, C], MDT, kind="ExternalInput")
    wp2 = nc.dram_tensor("wp2", [P, PAIRS, C], MDT, kind="ExternalInput")
    bias = nc.dram_tensor("bias", [1, C], F32, kind="ExternalInput")
    out = nc.dram_tensor("out", [QPC, C], F32, kind="ExternalOutput")
    with tile.TileContext(nc) as tc:
        with ExitStack() as ctx:
            if reps == 1:
                _emit(ctx, tc, nc, xT, xqT, wq, wk, wv, wp2, bias, out)
            else:
                # benchmark-only loop: branch-prefetch hints for the engines
                # whose bodies exceed one IRAM block
                with tc.For_i(
                    0, reps, 1,
                    hint_engines=(mybir.EngineType.PE, mybir.EngineType.Activation),
                ):
                    _emit(ctx, tc, nc, xT, xqT, wq, wk, wv, wp2, bias, out)
    nc.compile()
    return nc


_NC = None


def _get_nc():
    global _NC
    if _NC is None:
        _NC = build_nc()
    return _NC


def make_in_maps(x, w_qkv, w_proj, b_proj):
    x = np.asarray(x, np.float32)
    w_qkv = np.asarray(w_qkv, np.float32)
    w_proj = np.asarray(w_proj, np.float32)
    b_proj = np.asarray(b_proj, np.float32)

    wq = np.ascontiguousarray(
        (w_qkv[:, 0:C] * SCALE).reshape(3, P, C).transpose(1, 0, 2)
    )
    wk = np.ascontiguousarray(w_qkv[:, C:2 * C].reshape(3, P, C).transpose(1, 0, 2))
    wv = np.ascontiguousarray(w_qkv[:, 2 * C:3 * C].reshape(3, P, C).transpose(1, 0, 2))
    # head pairs stacked on the contraction dim: [128, PAIRS, C]
    wp2 = np.ascontiguousarray(
        w_proj.reshape(PAIRS, P, C).transpose(1, 0, 2)
    )
    bias = np.ascontiguousarray(b_proj.reshape(1, C))

    in_maps = []
    for core in range(NCORES):
        b, qi = core // 4, core % 4
        xT = np.ascontiguousarray(x[b].T)
        xq = np.ascontiguousarray(xT[:, qi * QPC:(qi + 1) * QPC])
        in_maps.append(
            {"xT": xT, "xqT": xq, "wq": wq, "wk": wk, "wv": wv, "wp2": wp2,
             "bias": bias}
        )
    return in_maps


def run(x, w_qkv, w_proj, b_proj, **run_kwargs):
    nc = _get_nc()
    in_maps = make_in_maps(x, w_qkv, w_proj, b_proj)
    res = run_bass_kernel_spmd(nc, in_maps, core_ids=list(range(NCORES)), **run_kwargs)
    out = np.empty((B, N, C), np.float32)
    for core in range(NCORES):
        b, qi = core // 4, core % 4
        out[b, qi * QPC:(qi + 1) * QPC] = res.results[core]["out"]
    return out, res


def kernel(x, w_qkv, w_proj, b_proj):
    out, _ = run(x, w_qkv, w_proj, b_proj)
    return out


# revision 40
# speedup vs baseline: 2.1346x; 2.1346x over previous
"""Multi-head self-attention (B=2, N=4096, C=384, H=6) on 8 Trainium2 NeuronCores.

Sharding: core = (batch, query-quarter). Each core recomputes K/V for its batch
from x (no cross-core communication), computes Q for its 1024 query rows, runs
softmax(Q K^T / sqrt(D)) V for all 6 heads, and applies the output projection
for its rows. Host concatenates the 8 row-slices.

Key device-side choices:
  - Everything is kept "transposed" (channel on partitions) so the PE never
    needs an on-chip transpose.
  - Heads are processed in pairs; the two K=64 score matmuls of a pair are
    row-tiled (partitions 0:64 / 64:128) so they can run concurrently in
    different PE row-groups.
  - softmax exp is computed as a Schraudolph bit-trick on TWO engines (ACT
    via Copy-activation with scale/bias, DVE via tensor_scalar mult+add,
    ~18:14 over the 32 score groups of each (pair, q-tile) window; Pool
    cannot read PSUM so it only gets the SBUF-side normalization multiplies):
        i16 = round_i16(s * (log2e*128) + B16);  bitcast(i16) == bf16(~e^s)
    Max per-element error ~3.3%, but the error is a deterministic function of
    frac(s*log2e) shared by numerator and denominator of the softmax, so the
    end-to-end output error stays ~8e-3 (measured vs fp64 reference).
    This halves the ACT time (the old exp bottleneck).
  - V carries an appended ones-column per head, so the P@V matmul also
    produces the softmax denominator for free (row 64 of O^T).
  - V and exp(S) are bf16 operands; scores stay f32r (Q/K^T f32r).
  - O accumulators are copied PSUM->SBUF (Pool) immediately after the last
    accumulation so the PSUM banks recycle fast; the normalization chain
    (DVE reciprocal + DRAM-bounce partition-broadcast + DVE multiply) runs
    entirely off the PE queue.
  - The normalized per-head outputs are written into per-PAIR [128, QPC]
    tiles so the output projection contracts K=128 (3 accumulation steps
    instead of 6).
"""

import ml_dtypes
import numpy as np
from contextlib import ExitStack

import concourse.bass as bass
import concourse.bacc as bacc
import concourse.tile as tile
from concourse import mybir
from concourse.bass_utils import run_bass_kernel_spmd

B, N, C = 2, 4096, 384
H, D = 6, 64
SCALE = D ** -0.5
P = 128
QPC = 1024          # query rows per core
NCORES = 8
PAIRS = H // 2      # 3 head pairs
NT = N // P         # 32 k-tiles
MDT = mybir.dt.float32r
F32 = mybir.dt.float32
BF16 = mybir.dt.bfloat16
I16 = mybir.dt.int16
EXPC = mybir.ActivationFunctionType.Copy

LOG2E = float(np.log2(np.e))
A16 = LOG2E * 128.0
B16 = 16250.375
# exp-engine schedule over the 32 score groups of one (pair, qt) window:
# A=ACT, D=DVE (Pool cannot read PSUM, so only these two engines can consume
# score tiles). ~4:3 matches the engines' effective elem rates (ACT ~126 G/s
# vs DVE ~110 G/s incl. its normalization work). The window STARTS with two
# ACT groups: the previous q-tile's normalization chain (reciprocal + DRAM
# bounce + multiply) is still draining in the in-order DVE queue, and a DVE
# exp op queued behind it would stall the PE's P@V stream.
_NA, _ND = 18, 14
EXP_WINDOW = "AA" + "".join(
    "AD"[(i * _ND) % 30 < _ND] for i in range(30)
)
assert EXP_WINDOW.count("A") == _NA and len(EXP_WINDOW) == 32
# engine choices (bisection knobs; defaults are the intended design)
V_COPY_ENGINE = "act"   # "act" | "dve"
MUL_ENGINE = "pool"     # "pool" | "dve"
RECIP_FAST = False
SPLIT_PV = True         # split P@V into two K=64 row-tiled halves


def _emit(ctx: ExitStack, tc, nc, xT, xqT, wq, wk, wv, wp2, bias, out):
    sing = ctx.enter_context(tc.tile_pool(name="sing", bufs=1))
    ktp = ctx.enter_context(tc.tile_pool(name="ktp", bufs=1))
    vp = ctx.enter_context(tc.tile_pool(name="vp", bufs=1))
    prep = ctx.enter_context(tc.tile_pool(name="prep", bufs=1))
    outp = ctx.enter_context(tc.tile_pool(name="outp", bufs=2))
    drp = ctx.enter_context(tc.tile_pool(name="drp", bufs=4, space="DRAM"))
    # PSUM pools are phase-scoped (stack discipline): kvgen/Q-gen use psp,
    # attention uses ssp (3 score bufs) + osp, proj re-creates a small pool.
    xchp = ctx.enter_context(tc.tile_pool(name="xchp", bufs=3))
    expp = ctx.enter_context(tc.tile_pool(name="expp", bufs=4))
    rbp = ctx.enter_context(tc.tile_pool(name="rbp", bufs=3))
    pctx = ExitStack()
    psp = pctx.enter_context(tc.tile_pool(name="psp", bufs=4, space="PSUM"))
    # created last / released right after Q^T generation (stack discipline)
    qctx = ExitStack()
    qtmp = qctx.enter_context(tc.tile_pool(name="qtmp", bufs=1))

    # ---- load weights / per-core query slice ----
    # DMA issue order: Q-gen's operands (wq, xq) first so the PE's first work
    # (Q-gen) starts as early as possible; kvgen's weights next (kvgen starts
    # right after Q-gen); wp/bias are only needed at the very end.
    wq_sb = qtmp.tile([P, 3, C], BF16, name="wq_sb")
    wk_sb = sing.tile([P, 3, C], BF16, name="wk_sb")
    wv_sb = sing.tile([P, 3, C], BF16, name="wv_sb")
    wp_sb = sing.tile([P, PAIRS, C], MDT, name="wp_sb")
    xq_sb = qtmp.tile([P, 3, QPC], BF16, name="xq_sb")
    nc.sync.dma_start(out=wq_sb, in_=wq[:, :, :])
    nc.sync.dma_start(
        out=xq_sb, in_=xqT[:, :].rearrange("(ck p) q -> p ck q", p=P)
    )
    nc.sync.dma_start(out=wv_sb, in_=wv[:, :, :])
    nc.sync.dma_start(out=wk_sb, in_=wk[:, :, :])
    nc.sync.dma_start(out=wp_sb, in_=wp2[:, :, :])
    bias_bc = sing.tile([P, C], F32, name="bias_bc")
    b_ap = bias[:, :]
    nc.sync.dma_start(
        out=bias_bc, in_=bass.AP(b_ap.tensor, b_ap.offset, [[0, P], [1, C]])
    )

    qt_sb = sing.tile([P, PAIRS, QPC], MDT, name="qt_sb")

    # ---- Q^T for all pairs: QT[:, pair, q] = (wq_pair)^T @ xq ----
    for pair in range(PAIRS):
        for qt in range(QPC // 512):
            ps = psp.tile([P, 512], F32, name="ps")
            for ck in range(3):
                nc.tensor.matmul(
                    ps,
                    lhsT=wq_sb[:, ck, pair * 128:(pair + 1) * 128],
                    rhs=xq_sb[:, ck, qt * 512:(qt + 1) * 512],
                    start=(ck == 0),
                    stop=(ck == 2),
                )
            nc.vector.tensor_copy(qt_sb[:, pair, qt * 512:(qt + 1) * 512], ps)
    qctx.close()  # wq/xq SBUF space is no longer needed

    # per-PAIR normalized outputs, heads stacked on partitions (proj K=128)
    pre2 = [prep.tile([P, QPC], MDT, name=f"pre{p}") for p in range(PAIRS)]

    xT_r = xT[:, :].rearrange("(ck p) n -> p ck n", p=P)

    # ---- V for ALL pairs (with ones columns), one xT streaming pass ----
    v_all = vp.tile([P, PAIRS, NT, 130], BF16, name="v_all")
    nc.vector.memset(v_all[:, :, :, 64:65], 1.0)
    nc.vector.memset(v_all[:, :, :, 129:130], 1.0)

    def kvgen(kt_tiles, nt8s):
        """One xT streaming pass computing V (all pairs) and K^T (all pairs).
        PE-bound (~3.8us/chunk vs ~2.4us DMA), so prefetch hides the DMA.
        V copies ride the ACT engine, K^T copies the DVE."""
        for nt8 in nt8s:
            xch = xchp.tile([P, 3, 512], BF16, name="xch")
            nc.sync.dma_start(out=xch, in_=xT_r[:, :, nt8 * 512:(nt8 + 1) * 512])
            for sub in range(4):
                nt = nt8 * 4 + sub
                psv = psp.tile([P, 512], F32, name="ps")[:, 0:C]
                for ck in range(3):
                    nc.tensor.matmul(
                        psv,
                        lhsT=xch[:, ck, sub * 128:(sub + 1) * 128],
                        rhs=wv_sb[:, ck, :],
                        start=(ck == 0),
                        stop=(ck == 2),
                    )
                # one copy for all pairs: psv [3 pairs x 2 heads x 64] ->
                # v_all column blocks (0:64 / 65:129 per pair). On ACT: it is
                # idle during kvgen and Pool cannot read PSUM.
                vcopy = nc.scalar.copy if V_COPY_ENGINE == "act" else nc.vector.tensor_copy
                vcopy(
                    v_all[:, :, nt, 0:130]
                    .rearrange("p r (two x) -> p r two x", two=2)[:, :, :, 0:64],
                    psv.rearrange("p (r two x) -> p r two x", r=PAIRS, two=2),
                )
            for p in range(PAIRS):
                ps = psp.tile([P, 512], F32, name="ps")
                for ck in range(3):
                    nc.tensor.matmul(
                        ps,
                        lhsT=wk_sb[:, ck, p * 128:(p + 1) * 128],
                        rhs=xch[:, ck, :],
                        start=(ck == 0),
                        stop=(ck == 2),
                    )
                nc.vector.tensor_copy(
                    kt_tiles[p][:, nt8 * 512:(nt8 + 1) * 512], ps
                )

    GRP = 2  # score chunks (512 q-cols each) per PSUM score tile / exp op

    def exp_op(idx, e_t, s_t, nelem):
        eng = EXP_WINDOW[idx % len(EXP_WINDOW)]
        if eng == "A":
            nc.scalar.activation(
                e_t[:, 0:nelem], s_t[:, 0:nelem], EXPC, bias=B16, scale=A16
            )
        else:
            nc.vector.tensor_scalar(
                e_t[:, 0:nelem], s_t[:, 0:nelem], A16, B16,
                mybir.AluOpType.mult, mybir.AluOpType.add,
            )

    def attention(pair, kt_t):
        NCH = 2 * NT  # chunk c = (kt = c//2, half = c%2)

        def s_chunk(s_t, j, c, qt):
            kt, half = c // 2, c % 2
            lo = 64 * half
            nc.tensor.matmul(
                s_t[:, j * 512:(j + 1) * 512],
                lhsT=kt_t[lo:lo + 64, kt * 128:(kt + 1) * 128],
                rhs=qt_sb[lo:lo + 64, pair, qt * 512:(qt + 1) * 512],
                start=True,
                stop=True,
            )

        def s_group(g, qt):
            nch = min(GRP, NCH - g * GRP)
            s_t = ssp.tile([P, GRP * 512], F32, name="s")
            for j in range(nch):
                s_chunk(s_t, j, g * GRP + j, qt)
            return s_t, nch

        for qt in range(QPC // 512):
            # P@V is split into two K=64 row-tiled halves (partitions 0:64 /
            # 64:128) accumulating into separate PSUM banks: the halves run
            # concurrently in different PE row-groups (their rhs streams live
            # on disjoint SBUF partition halves), halving P@V wall time. A
            # DVE add merges the partials during normalization.
            o_acc = [
                [osp.tile([65, 512], F32, name=f"o{h}{ab}") for ab in "ab"]
                for h in range(2)
            ]
            ngrp = (NCH + GRP - 1) // GRP
            # score matmuls run two groups ahead of exp
            s_ts = [s_group(0, qt), s_group(1, qt)]
            for g in range(ngrp):
                s_t, nch = s_ts.pop(0)
                e_t = expp.tile([P, GRP * 512], I16, name="etile")
                exp_op(g, e_t, s_t, nch * 512)
                if g + 2 < ngrp:
                    s_ts.append(s_group(g + 2, qt))
                for j in range(nch):
                    c = g * GRP + j
                    kt, half = c // 2, c % 2
                    lo, hi = (65, 130) if half else (0, 65)
                    e_bf = e_t[:, j * 512:(j + 1) * 512].bitcast(BF16)
                    if SPLIT_PV:
                        for ab in range(2):
                            nc.tensor.matmul(
                                o_acc[half][ab],
                                lhsT=v_all[ab * 64:(ab + 1) * 64, pair, kt, lo:hi],
                                rhs=e_bf[ab * 64:(ab + 1) * 64, :],
                                start=(kt == 0),
                                stop=(kt == NT - 1),
                            )
                    else:
                        nc.tensor.matmul(
                            o_acc[half][0],
                            lhsT=v_all[:, pair, kt, lo:hi],
                            rhs=e_bf,
                            start=(kt == 0),
                            stop=(kt == NT - 1),
                        )
            # normalize: pre2[pair][h*64:(h+1)*64, q] = O^T[0:64, q] / O^T[64, q]
            for hh, (oA, oB) in ((0, o_acc[0]), (1, o_acc[1])):
                # merge the two K-halves off PSUM (frees the banks), then
                # reciprocal + DRAM-bounce partition-broadcast + Pool multiply.
                # DVE cannot read two PSUM operands, so ACT stages one half
                # to SBUF first.
                osum = rbp.tile([65, 512], F32, name="osum")
                if SPLIT_PV:
                    ob_sb = rbp.tile([65, 512], F32, name="obsb")
                    nc.scalar.copy(ob_sb, oB)
                    nc.vector.tensor_add(osum, oA, ob_sb)
                else:
                    nc.scalar.copy(osum, oA)
                # the denominator row lives on ONE partition; reciprocal is
                # iterative-divide (~6 cyc/elem) so a [1,512] op wastes 127
                # lanes. Bounce the row to DRAM and read it back partition-
                # major as [128,4]: same reciprocal, 128x fewer cycles. The
                # result bounces back out for the [64,512] partition-broadcast
                # read (step-0 partition APs are only legal on DRAM).
                d_dr = drp.tile([1, 512], F32, name="ddr")
                nc.sync.dma_start(out=d_dr, in_=osum[64:65, :])
                dT = rbp.tile([P, 4], F32, name="dT")
                dap = d_dr[:, :]
                nc.sync.dma_start(
                    out=dT, in_=bass.AP(dap.tensor, dap.offset, [[4, P], [1, 4]])
                )
                recipT = rbp.tile([P, 4], F32, name="recipT")
                nc.vector.reciprocal(recipT, dT)
                rdr = drp.tile([1, 512], F32, name="rdr")
                rap = rdr[:, :]
                nc.sync.dma_start(
                    out=bass.AP(rap.tensor, rap.offset, [[4, P], [1, 4]]),
                    in_=recipT,
                )
                rb_sb = rbp.tile([64, 512], F32, name="rb")
                nc.sync.dma_start(
                    out=rb_sb,
                    in_=bass.AP(rap.tensor, rap.offset, [[0, 64], [1, 512]]),
                )
                mul = nc.gpsimd.tensor_mul if MUL_ENGINE == "pool" else nc.vector.tensor_mul
                mul(
                    pre2[pair][hh * 64:(hh + 1) * 64, qt * 512:(qt + 1) * 512],
                    osum[0:64, :], rb_sb
                )

    kt_tiles = [ktp.tile([P, N], MDT, name=f"kt{pair}") for pair in range(PAIRS)]
    kvgen(kt_tiles, range(N // 512))
    pctx.close()  # free kvgen PSUM banks for the attention pools

    actx = ExitStack()
    ssp = actx.enter_context(tc.tile_pool(name="ssp", bufs=2, space="PSUM"))
    osp = actx.enter_context(tc.tile_pool(name="osp", bufs=1, space="PSUM"))
    for pair in range(PAIRS):
        attention(pair, kt_tiles[pair])
    actx.close()

    psp = ctx.enter_context(tc.tile_pool(name="psp2", bufs=2, space="PSUM"))

    # ---- output projection: contract K=128 over head pairs ----
    for qc in range(QPC // P):
        ps = psp.tile([P, 512], F32, name="ps")[:, 0:C]
        for pair in range(PAIRS):
            nc.tensor.matmul(
                ps,
                lhsT=pre2[pair][:, qc * P:(qc + 1) * P],
                rhs=wp_sb[:, pair, :],
                start=(pair == 0),
                stop=(pair == PAIRS - 1),
            )
        o_sb = outp.tile([P, C], F32, name="osb")
        nc.vector.tensor_add(o_sb, ps, bias_bc)
        nc.sync.dma_start(out=out[qc * P:(qc + 1) * P, :], in_=o_sb)


def build_nc(reps=1):
    nc = bacc.Bacc()
    # x and the qkv-generation weights stream in bf16: halves HBM traffic
    # (the dominant DMA stream) at ~0.4% input quantization, far inside the
    # error budget. Scores/P@V paths keep f32r/bf16 as before.
    xT = nc.dram_tensor("xT", [C, N], BF16, kind="ExternalInput")
    xqT = nc.dram_tensor("xqT", [C, QPC], BF16, kind="ExternalInput")
    wq = nc.dram_tensor("wq", [P, 3, C], BF16, kind="ExternalInput")
    wk = nc.dram_tensor("wk", [P, 3, C], BF16, kind="ExternalInput")
    wv = nc.dram_tensor("wv", [P, 3, C], BF16, kind="ExternalInput")
    wp2 = nc.dram_tensor("wp2", [P, PAIRS, C], MDT, kind="ExternalInput")
    bias = nc.dram_tensor("bias", [1, C], F32, kind="ExternalInput")
    out = nc.dram_tensor("out", [QPC, C], F32, kind="ExternalOutput")
    with tile.TileContext(nc) as tc:
        with ExitStack() as ctx:
            if reps == 1:
                _emit(ctx, tc, nc, xT, xqT, wq, wk, wv, wp2, bias, out)
            else:
                # benchmark-only loop: branch-prefetch hints for the engines
                # whose bodies exceed one IRAM block
                with tc.For_i(
                    0, reps, 1,
                    hint_engines=(mybir.EngineType.PE, mybir.EngineType.Activation),
                ):
                    _emit(ctx, tc, nc, xT, xqT, wq, wk, wv, wp2, bias, out)
    nc.compile()
    return nc


_NC = None


def _get_nc():
    global _NC
    if _NC is None:
        _NC = build_nc()
    return _NC


def make_in_maps(x, w_qkv, w_proj, b_proj):
    x = np.asarray(x, np.float32)
    w_qkv = np.asarray(w_qkv, np.float32)
    w_proj = np.asarray(w_proj, np.float32)
    b_proj = np.asarray(b_proj, np.float32)

    bf16 = ml_dtypes.bfloat16
    wq = np.ascontiguousarray(
        (w_qkv[:, 0:C] * SCALE).reshape(3, P, C).transpose(1, 0, 2)
    ).astype(bf16)
    wk = np.ascontiguousarray(
        w_qkv[:, C:2 * C].reshape(3, P, C).transpose(1, 0, 2)
    ).astype(bf16)
    wv = np.ascontiguousarray(
        w_qkv[:, 2 * C:3 * C].reshape(3, P, C).transpose(1, 0, 2)
    ).astype(bf16)
    # head pairs stacked on the contraction dim: [128, PAIRS, C]
    wp2 = np.ascontiguousarray(
        w_proj.reshape(PAIRS, P, C).transpose(1, 0, 2)
    )
    bias = np.ascontiguousarray(b_proj.reshape(1, C))

    in_maps = []
    for core in range(NCORES):
        b, qi = core // 4, core % 4
        xT = np.ascontiguousarray(x[b].T).astype(bf16)
        xq = np.ascontiguousarray(xT[:, qi * QPC:(qi + 1) * QPC])
        in_maps.append(
            {"xT": xT, "xqT": xq, "wq": wq, "wk": wk, "wv": wv, "wp2": wp2,
             "bias": bias}
        )
    return in_maps


def run(x, w_qkv, w_proj, b_proj, **run_kwargs):
    nc = _get_nc()
    in_maps = make_in_maps(x, w_qkv, w_proj, b_proj)
    res = run_bass_kernel_spmd(nc, in_maps, core_ids=list(range(NCORES)), **run_kwargs)
    out = np.empty((B, N, C), np.float32)
    for core in range(NCORES):
        b, qi = core // 4, core % 4
        out[b, qi * QPC:(qi + 1) * QPC] = res.results[core]["out"]
    return out, res


def kernel(x, w_qkv, w_proj, b_proj):
    out, _ = run(x, w_qkv, w_proj, b_proj)
    return out
